# revision 11
# baseline (speedup 1.0000x reference)
"""Trainium2 Bass kernel for nn_ContextAttentionBlock.

Reference computation (per row-strip n of B*H = 2048, each strip [W=128, C=128]):
    ft = x @ Wt, fp = x @ Wp, fg = x @ Wg           (1x1 convs, biases are zero)
    h[w,v] = sum_c fp[w,c] ft[v,c]   -> h_res = sigmoid(h) * x
    v[c,d] = sum_w fg[w,c] fp[w,d]   -> v_res = sigmoid(v) * x
    sc     = x @ Wsc
    out    = [h_res | sc | v_res] @ Wout

Kernel algebra (host-precomputed constants fold two matmuls):
    M1      = Wp @ Wt.T          => h = x @ M1 @ x.T
    Wsc_out = Wsc @ Wout[128:256] => sc contribution = x @ Wsc_out
    Per strip on-device (all 128x128x128-class matmuls on PE):
      xT   = transpose(x)                      (PE transpose via identity)
      r    = M1.T @ x.T     = matmul(lhsT=M1, rhs=xT)       [batched over 4 strips]
      hT   = x @ r          = matmul(lhsT=xT, rhs=r)
      fp|fg= x @ [Wp|Wg]    = matmul(lhsT=xT, rhs=WpWg)
      vT   = fp.T @ fg      = matmul(lhsT=fp, rhs=fg)
      out  = x @ Wsc_out + (sig(hT)*xT).T-matmul Wh + (sig(vT)*xT) Wv  (PSUM accum)

Sharding: data-parallel over B*H across 8 cores (256 strips/core); weights
replicated on all cores.
"""

import os
import sys

sys.path.insert(0, "/opt/trn_rl_repo")

import numpy as np
import ml_dtypes

import concourse.bass as bass
import concourse.bacc as bacc
import concourse.mybir as mybir
from concourse.tile import TileContext
from concourse.tile_rust import add_dep_helper
from concourse.bass_utils import run_bass_kernel_spmd

N_CORES = 8
BH = 2048
SPC = BH // N_CORES  # strips per core
W = 128
C = 128
GROUP = 4  # strips per group (r-matmul batch)

# 'bf16' (fast) or 'f32' (exact, 4x slower matmuls)
VARIANT = os.environ.get("CAB_VARIANT", "f32")
TRACE = os.environ.get("CAB_TRACE", "0") == "1"
# Repeat the whole workload inside the NEFF (for device-time measurement via
# wall-clock deltas; results are identical for any repeat count).
REPEAT = int(os.environ.get("CAB_REPEAT", "1"))

last_results = None  # BassKernelResults from the most recent run (for test.py)

_nc_cache = {}


def _build(variant: str, repeat: int = 1) -> bass.Bass:
    f32 = mybir.dt.float32
    cdt = mybir.dt.bfloat16 if variant == "bf16" else f32

    nc = bacc.Bacc("TRN2", target_bir_lowering=False, debug=False)
    x_in = nc.declare_dram_parameter("x", [SPC, W, C], cdt, False)
    ident_in = nc.declare_dram_parameter("ident", [C, C], cdt, False)
    m1_in = nc.declare_dram_parameter("m1", [C, C], cdt, False)
    wpg_in = nc.declare_dram_parameter("wpg", [C, 2 * C], cdt, False)
    wsc_in = nc.declare_dram_parameter("wsc", [C, C], cdt, False)
    whv_in = nc.declare_dram_parameter("whv", [C, 2 * C], cdt, False)
    out_d = nc.declare_dram_parameter("out", [SPC, W, C], f32, True)

    sig_f = mybir.ActivationFunctionType.Sigmoid

    def chain(prev, inst):
        # Keep matmuls of a shared-bank accumulation group in program order.
        if prev is not None:
            add_dep_helper(inst.ins, prev.ins, sync=False,
                           reason="psum group order")
        return inst

    with TileContext(nc) as tc:
        with (
            tc.tile_pool(name="const", bufs=1) as constp,
            tc.tile_pool(name="sb", bufs=3) as sb,
            tc.tile_pool(name="ps", bufs=2, space="PSUM") as ps,
        ):
            ident_sb = constp.tile([C, C], cdt)
            nc.sync.dma_start(out=ident_sb, in_=ident_in[:, :])
            m1_sb = constp.tile([C, C], cdt)
            nc.sync.dma_start(out=m1_sb, in_=m1_in[:, :])
            wpg_sb = constp.tile([C, 2 * C], cdt)
            nc.sync.dma_start(out=wpg_sb, in_=wpg_in[:, :])
            wsc_sb = constp.tile([C, C], cdt)
            nc.sync.dma_start(out=wsc_sb, in_=wsc_in[:, :])
            whv_sb = constp.tile([C, 2 * C], cdt)
            nc.sync.dma_start(out=whv_sb, in_=whv_in[:, :])

            for g0 in [
                g for _ in range(repeat) for g in range(0, SPC, GROUP)
            ]:
                # ---- load 4 strips: [W, 4*C], strip-major in free dim
                x4 = sb.tile([W, GROUP * C], cdt, tag="x4")
                nc.sync.dma_start(
                    out=x4.rearrange("w (g c) -> w g c", g=GROUP),
                    in_=x_in[g0 : g0 + GROUP].rearrange("g w c -> w g c"),
                )

                # ---- transpose each strip on PE: xT4 = [xT_0 | xT_1 | xT_2 | xT_3]
                xT_ps = ps.tile([C, GROUP * W], cdt, tag="xT_ps", bufs=1)
                tprev = None
                for g in range(GROUP):
                    tprev = chain(tprev, nc.tensor.matmul(
                        xT_ps[:, g * W : (g + 1) * W],
                        lhsT=x4[:, g * C : (g + 1) * C],
                        rhs=ident_sb[:, :],
                        is_transpose=True,
                        start=(g == 0),
                        stop=(g == GROUP - 1),
                    ))
                xT = sb.tile([C, GROUP * W], cdt, tag="xT")
                nc.scalar.copy(out=xT, in_=xT_ps)

                # ---- r4 = M1^T @ [xT_0..xT_3]  (shared stationary M1)
                r4_ps = ps.tile([C, GROUP * W], f32, tag="r4_ps", bufs=1)
                nc.tensor.matmul(r4_ps, lhsT=m1_sb, rhs=xT, start=True, stop=True)
                r4 = sb.tile([C, GROUP * W], cdt, tag="r4")
                nc.vector.tensor_copy(out=r4, in_=r4_ps)

                # ---- per-group output accumulator (one PSUM bank, 4 strips)
                # All 12 matmuls into this bank form ONE accumulation group
                # (PSUM start/stop is bank-granular): sc_0 opens, E_3 closes.
                out4_ps = ps.tile([W, GROUP * C], f32, tag="out4", bufs=2)
                oprev = None
                n_out_mms = 3 * GROUP
                out_mm_idx = 0

                for p in range(GROUP // 2):  # pairs of strips
                    pair = (2 * p, 2 * p + 1)
                    # P1 = [hT_a | vT_a | hT_b | vT_b]; one group per bank
                    p1 = ps.tile([W, 4 * C], f32, tag="p1", bufs=2)
                    fps = ps.tile([W, 4 * C], f32, tag="fps", bufs=2)
                    pprev = None
                    fprev = None
                    for i, s in enumerate(pair):
                        xTs = xT[:, s * W : (s + 1) * W]
                        # hT = x @ r
                        pprev = chain(pprev, nc.tensor.matmul(
                            p1[:, (2 * i) * C : (2 * i + 1) * C],
                            lhsT=xTs,
                            rhs=r4[:, s * W : (s + 1) * W],
                            start=(i == 0),
                            stop=False,
                        ))
                        # [fp | fg] = x @ [Wp | Wg]
                        fprev = chain(fprev, nc.tensor.matmul(
                            fps[:, (2 * i) * C : (2 * i + 2) * C],
                            lhsT=xTs,
                            rhs=wpg_sb,
                            start=(i == 0),
                            stop=(i == 1),
                        ))
                        # shortcut contribution: out += x @ Wsc_out
                        oprev = chain(oprev, nc.tensor.matmul(
                            out4_ps[:, s * C : (s + 1) * C],
                            lhsT=xTs,
                            rhs=wsc_sb,
                            start=(out_mm_idx == 0),
                            stop=(out_mm_idx == n_out_mms - 1),
                        ))
                        out_mm_idx += 1
                    f_sb = sb.tile([W, 4 * C], cdt, tag="f_sb")
                    nc.vector.tensor_copy(out=f_sb, in_=fps)
                    for i, s in enumerate(pair):
                        # vT = fp^T @ fg
                        pprev = chain(pprev, nc.tensor.matmul(
                            p1[:, (2 * i + 1) * C : (2 * i + 2) * C],
                            lhsT=f_sb[:, (2 * i) * C : (2 * i + 1) * C],
                            rhs=f_sb[:, (2 * i + 1) * C : (2 * i + 2) * C],
                            start=False,
                            stop=(i == 1),
                        ))
                    # sigmoid over the whole pair tile [hT_a|vT_a|hT_b|vT_b]
                    sig = sb.tile([W, 4 * C], cdt, tag="sig")
                    nc.scalar.activation(sig, p1, sig_f)
                    # res = sig * [xT_a | xT_a | xT_b | xT_b]
                    res = sb.tile([W, 4 * C], cdt, tag="res")
                    xp = xT[:, 2 * p * W : (2 * p + 2) * W]
                    x_b = bass.AP(
                        tensor=xp.tensor,
                        offset=xp.offset,
                        ap=[xp.ap[0], [W, 2], [0, 2], [1, W]],
                    )
                    sig4 = sig.rearrange("p (a r c) -> p a r c", a=2, r=2)
                    res4 = res.rearrange("p (a r c) -> p a r c", a=2, r=2)
                    if p % 2 == 0:
                        nc.vector.tensor_mul(res4, sig4, x_b)
                    else:
                        nc.gpsimd.tensor_mul(res4, sig4, x_b)
                    for i, s in enumerate(pair):
                        oprev = chain(oprev, nc.tensor.matmul(
                            out4_ps[:, s * C : (s + 1) * C],
                            lhsT=res[:, (2 * i) * C : (2 * i + 1) * C],
                            rhs=whv_sb[:, 0:C],
                            start=(out_mm_idx == 0),
                            stop=(out_mm_idx == n_out_mms - 1),
                        ))
                        out_mm_idx += 1
                        oprev = chain(oprev, nc.tensor.matmul(
                            out4_ps[:, s * C : (s + 1) * C],
                            lhsT=res[:, (2 * i + 1) * C : (2 * i + 2) * C],
                            rhs=whv_sb[:, C : 2 * C],
                            start=(out_mm_idx == 0),
                            stop=(out_mm_idx == n_out_mms - 1),
                        ))
                        out_mm_idx += 1

                out_sb = sb.tile([W, GROUP * C], f32, tag="out_sb")
                nc.scalar.copy(out=out_sb, in_=out4_ps)
                nc.sync.dma_start(
                    out=out_d[g0 : g0 + GROUP].rearrange("g w c -> w g c"),
                    in_=out_sb.rearrange("w (g c) -> w g c", g=GROUP),
                )
    nc.compile()
    return nc


def _get_nc(variant: str, repeat: int = 1) -> bass.Bass:
    key = (variant, repeat)
    if key not in _nc_cache:
        _nc_cache[key] = _build(variant, repeat)
    return _nc_cache[key]


def kernel(
    x,
    w_theta,
    b_theta,
    w_phi,
    b_phi,
    w_g,
    b_g,
    w_sc,
    b_sc,
    w_out,
    b_out,
):
    global last_results
    x = np.asarray(x, dtype=np.float32)
    w_theta = np.asarray(w_theta, dtype=np.float32)
    w_phi = np.asarray(w_phi, dtype=np.float32)
    w_g = np.asarray(w_g, dtype=np.float32)
    w_sc = np.asarray(w_sc, dtype=np.float32)
    w_out = np.asarray(w_out, dtype=np.float32)
    b_theta = np.asarray(b_theta, dtype=np.float32)
    b_phi = np.asarray(b_phi, dtype=np.float32)
    b_g = np.asarray(b_g, dtype=np.float32)
    b_sc = np.asarray(b_sc, dtype=np.float32)
    b_out = np.asarray(b_out, dtype=np.float32)

    # The attention-path biases are structurally zero for this problem; the
    # shortcut/output biases fold into a host-side constant row added at the end.
    assert not b_theta.any() and not b_phi.any() and not b_g.any(), (
        "kernel assumes zero theta/phi/g biases"
    )

    B, H, Wd, Cd = x.shape
    assert (B * H, Wd, Cd) == (BH, W, C)

    m1 = w_phi @ w_theta.T
    wsc_out = w_sc @ w_out[C : 2 * C]
    wpg = np.concatenate([w_phi, w_g], axis=1)
    whv = np.concatenate([w_out[0:C], w_out[2 * C : 3 * C]], axis=1)
    ident = np.eye(C, dtype=np.float32)
    bias_row = b_out + b_sc @ w_out[C : 2 * C]  # exact fold of b_sc and b_out

    variant = VARIANT
    np_dt = ml_dtypes.bfloat16 if variant == "bf16" else np.float32
    xs = x.reshape(BH, W, C).astype(np_dt)
    consts = {
        "ident": ident.astype(np_dt),
        "m1": m1.astype(np_dt),
        "wpg": wpg.astype(np_dt),
        "wsc": wsc_out.astype(np_dt),
        "whv": whv.astype(np_dt),
    }
    in_maps = [
        {"x": np.ascontiguousarray(xs[i * SPC : (i + 1) * SPC]), **consts}
        for i in range(N_CORES)
    ]

    nc = _get_nc(variant, REPEAT)
    try:
        last_results = run_bass_kernel_spmd(
            nc, in_maps, core_ids=list(range(N_CORES)), trace=TRACE
        )
    except ModuleNotFoundError:
        # axon NTFF profiling hook unavailable in this environment
        last_results = run_bass_kernel_spmd(
            nc, in_maps, core_ids=list(range(N_CORES)), trace=False
        )
    out = np.concatenate(
        [last_results.results[i]["out"] for i in range(N_CORES)], axis=0
    ).reshape(B, H, W, C)
    if bias_row.any():
        out = out + bias_row
    return out.astype(np.float32)


# revision 13
# speedup vs baseline: 94.0942x; 94.0942x over previous
"""Trainium2 Bass kernel for nn_ContextAttentionBlock.

Reference computation (per row-strip n of B*H = 2048, each strip [W=128, C=128]):
    ft = x @ Wt, fp = x @ Wp, fg = x @ Wg           (1x1 convs, biases are zero)
    h[w,v] = sum_c fp[w,c] ft[v,c]   -> h_res = sigmoid(h) * x
    v[c,d] = sum_w fg[w,c] fp[w,d]   -> v_res = sigmoid(v) * x
    sc     = x @ Wsc
    out    = [h_res | sc | v_res] @ Wout

Kernel algebra (host-precomputed constants fold two matmuls):
    M1      = Wp @ Wt.T          => h = x @ M1 @ x.T
    Wsc_out = Wsc @ Wout[128:256] => sc contribution = x @ Wsc_out
    Per strip on-device (all 128x128x128-class matmuls on PE):
      xT   = transpose(x)                      (PE transpose via identity)
      r    = M1.T @ x.T     = matmul(lhsT=M1, rhs=xT)       [batched over 4 strips]
      hT   = x @ r          = matmul(lhsT=xT, rhs=r)
      fp|fg= x @ [Wp|Wg]    = matmul(lhsT=xT, rhs=WpWg)
      vT   = fp.T @ fg      = matmul(lhsT=fp, rhs=fg)
      out  = x @ Wsc_out + (sig(hT)*xT).T-matmul Wh + (sig(vT)*xT) Wv  (PSUM accum)

Sharding: data-parallel over B*H across 8 cores (256 strips/core); weights
replicated on all cores.
"""

import os
import sys

sys.path.insert(0, "/opt/trn_rl_repo")

import numpy as np
import ml_dtypes

import concourse.bass as bass
import concourse.bacc as bacc
import concourse.mybir as mybir
from concourse.tile import TileContext
from concourse.tile_rust import add_dep_helper
from concourse.bass_utils import run_bass_kernel_spmd

N_CORES = 8
BH = 2048
SPC = int(os.environ.get("CAB_SPC", str(BH // N_CORES)))  # strips per core
W = 128
C = 128
GROUP = 4  # strips per group (r-matmul batch)

# 'bf16' (fast) or 'f32' (exact, 4x slower matmuls)
VARIANT = os.environ.get("CAB_VARIANT", "f32")
TRACE = os.environ.get("CAB_TRACE", "0") == "1"
# Repeat the whole workload inside the NEFF (for device-time measurement via
# wall-clock deltas; results are identical for any repeat count).
REPEAT = int(os.environ.get("CAB_REPEAT", "1"))

last_results = None  # BassKernelResults from the most recent run (for test.py)

_nc_cache = {}


def _build(variant: str, repeat: int = 1) -> bass.Bass:
    f32 = mybir.dt.float32
    cdt = mybir.dt.bfloat16 if variant == "bf16" else f32

    nc = bacc.Bacc("TRN2", target_bir_lowering=False, debug=False)
    x_in = nc.declare_dram_parameter("x", [SPC, W, C], cdt, False)
    ident_in = nc.declare_dram_parameter("ident", [C, C], cdt, False)
    m1_in = nc.declare_dram_parameter("m1", [C, C], cdt, False)
    wpg_in = nc.declare_dram_parameter("wpg", [C, 2 * C], cdt, False)
    wsc_in = nc.declare_dram_parameter("wsc", [C, C], cdt, False)
    whv_in = nc.declare_dram_parameter("whv", [C, 2 * C], cdt, False)
    out_d = nc.declare_dram_parameter("out", [SPC, W, C], f32, True)

    sig_f = mybir.ActivationFunctionType.Sigmoid

    def chain(prev, inst):
        # Keep matmuls of a shared-bank accumulation group in program order.
        if prev is not None:
            add_dep_helper(inst.ins, prev.ins, sync=False,
                           reason="psum group order")
        return inst

    with TileContext(nc) as tc:
        with (
            tc.tile_pool(name="const", bufs=1) as constp,
            tc.tile_pool(name="sb", bufs=3) as sb,
            tc.tile_pool(name="ps", bufs=2, space="PSUM") as ps,
        ):
            ident_sb = constp.tile([C, C], cdt)
            nc.sync.dma_start(out=ident_sb, in_=ident_in[:, :])
            m1_sb = constp.tile([C, C], cdt)
            nc.sync.dma_start(out=m1_sb, in_=m1_in[:, :])
            wpg_sb = constp.tile([C, 2 * C], cdt)
            nc.sync.dma_start(out=wpg_sb, in_=wpg_in[:, :])
            wsc_sb = constp.tile([C, C], cdt)
            nc.sync.dma_start(out=wsc_sb, in_=wsc_in[:, :])
            whv_sb = constp.tile([C, 2 * C], cdt)
            nc.sync.dma_start(out=whv_sb, in_=whv_in[:, :])

            for g0 in [
                g for _ in range(repeat) for g in range(0, SPC, GROUP)
            ]:
                # ---- load 4 strips: [W, 4*C], strip-major in free dim
                x4 = sb.tile([W, GROUP * C], cdt, tag="x4")
                nc.sync.dma_start(
                    out=x4.rearrange("w (g c) -> w g c", g=GROUP),
                    in_=x_in[g0 : g0 + GROUP].rearrange("g w c -> w g c"),
                )

                # ---- transpose each strip on PE: xT4 = [xT_0 | xT_1 | xT_2 | xT_3]
                xT_ps = ps.tile([C, GROUP * W], cdt, tag="xT_ps", bufs=1)
                tprev = None
                for g in range(GROUP):
                    tprev = chain(tprev, nc.tensor.matmul(
                        xT_ps[:, g * W : (g + 1) * W],
                        lhsT=x4[:, g * C : (g + 1) * C],
                        rhs=ident_sb[:, :],
                        is_transpose=True,
                        start=(g == 0),
                        stop=(g == GROUP - 1),
                    ))
                xT = sb.tile([C, GROUP * W], cdt, tag="xT")
                nc.scalar.copy(out=xT, in_=xT_ps)

                # ---- r4 = M1^T @ [xT_0..xT_3]  (shared stationary M1)
                r4_ps = ps.tile([C, GROUP * W], f32, tag="r4_ps", bufs=1)
                nc.tensor.matmul(r4_ps, lhsT=m1_sb, rhs=xT, start=True, stop=True)
                r4 = sb.tile([C, GROUP * W], cdt, tag="r4")
                nc.vector.tensor_copy(out=r4, in_=r4_ps)

                # ---- per-group output accumulator (one PSUM bank, 4 strips)
                # All 12 matmuls into this bank form ONE accumulation group
                # (PSUM start/stop is bank-granular): sc_0 opens, E_3 closes.
                out4_ps = ps.tile([W, GROUP * C], f32, tag="out4", bufs=2)
                oprev = None
                n_out_mms = 3 * GROUP
                out_mm_idx = 0

                for p in range(GROUP // 2):  # pairs of strips
                    pair = (2 * p, 2 * p + 1)
                    # P1 = [hT_a | vT_a | hT_b | vT_b]; one group per bank
                    p1 = ps.tile([W, 4 * C], f32, tag="p1", bufs=2)
                    fps = ps.tile([W, 4 * C], f32, tag="fps", bufs=2)
                    pprev = None
                    fprev = None
                    for i, s in enumerate(pair):
                        xTs = xT[:, s * W : (s + 1) * W]
                        # hT = x @ r
                        pprev = chain(pprev, nc.tensor.matmul(
                            p1[:, (2 * i) * C : (2 * i + 1) * C],
                            lhsT=xTs,
                            rhs=r4[:, s * W : (s + 1) * W],
                            start=(i == 0),
                            stop=False,
                        ))
                        # [fp | fg] = x @ [Wp | Wg]
                        fprev = chain(fprev, nc.tensor.matmul(
                            fps[:, (2 * i) * C : (2 * i + 2) * C],
                            lhsT=xTs,
                            rhs=wpg_sb,
                            start=(i == 0),
                            stop=(i == 1),
                        ))
                        # shortcut contribution: out += x @ Wsc_out
                        oprev = chain(oprev, nc.tensor.matmul(
                            out4_ps[:, s * C : (s + 1) * C],
                            lhsT=xTs,
                            rhs=wsc_sb,
                            start=(out_mm_idx == 0),
                            stop=(out_mm_idx == n_out_mms - 1),
                        ))
                        out_mm_idx += 1
                    f_sb = sb.tile([W, 4 * C], cdt, tag="f_sb")
                    nc.vector.tensor_copy(out=f_sb, in_=fps)
                    for i, s in enumerate(pair):
                        # vT = fp^T @ fg
                        pprev = chain(pprev, nc.tensor.matmul(
                            p1[:, (2 * i + 1) * C : (2 * i + 2) * C],
                            lhsT=f_sb[:, (2 * i) * C : (2 * i + 1) * C],
                            rhs=f_sb[:, (2 * i + 1) * C : (2 * i + 2) * C],
                            start=False,
                            stop=(i == 1),
                        ))
                    # sigmoid over the whole pair tile [hT_a|vT_a|hT_b|vT_b]
                    sig = sb.tile([W, 4 * C], cdt, tag="sig")
                    nc.scalar.activation(sig, p1, sig_f)
                    # res = sig * [xT_a | xT_a | xT_b | xT_b]
                    res = sb.tile([W, 4 * C], cdt, tag="res")
                    xp = xT[:, 2 * p * W : (2 * p + 2) * W]
                    x_b = bass.AP(
                        tensor=xp.tensor,
                        offset=xp.offset,
                        ap=[xp.ap[0], [W, 2], [0, 2], [1, W]],
                    )
                    sig4 = sig.rearrange("p (a r c) -> p a r c", a=2, r=2)
                    res4 = res.rearrange("p (a r c) -> p a r c", a=2, r=2)
                    mul_gps = os.environ.get("CAB_MUL_GPS", "1") == "1"
                    if p % 2 == 0 or not mul_gps:
                        nc.vector.tensor_mul(res4, sig4, x_b)
                    else:
                        nc.gpsimd.tensor_mul(res4, sig4, x_b)
                    for i, s in enumerate(pair):
                        oprev = chain(oprev, nc.tensor.matmul(
                            out4_ps[:, s * C : (s + 1) * C],
                            lhsT=res[:, (2 * i) * C : (2 * i + 1) * C],
                            rhs=whv_sb[:, 0:C],
                            start=(out_mm_idx == 0),
                            stop=(out_mm_idx == n_out_mms - 1),
                        ))
                        out_mm_idx += 1
                        oprev = chain(oprev, nc.tensor.matmul(
                            out4_ps[:, s * C : (s + 1) * C],
                            lhsT=res[:, (2 * i + 1) * C : (2 * i + 2) * C],
                            rhs=whv_sb[:, C : 2 * C],
                            start=(out_mm_idx == 0),
                            stop=(out_mm_idx == n_out_mms - 1),
                        ))
                        out_mm_idx += 1

                out_sb = sb.tile([W, GROUP * C], f32, tag="out_sb")
                nc.scalar.copy(out=out_sb, in_=out4_ps)
                nc.sync.dma_start(
                    out=out_d[g0 : g0 + GROUP].rearrange("g w c -> w g c"),
                    in_=out_sb.rearrange("w (g c) -> w g c", g=GROUP),
                )
    nc.compile()
    return nc


def _get_nc(variant: str, repeat: int = 1) -> bass.Bass:
    key = (variant, repeat)
    if key not in _nc_cache:
        _nc_cache[key] = _build(variant, repeat)
    return _nc_cache[key]


def kernel(
    x,
    w_theta,
    b_theta,
    w_phi,
    b_phi,
    w_g,
    b_g,
    w_sc,
    b_sc,
    w_out,
    b_out,
):
    global last_results
    x = np.asarray(x, dtype=np.float32)
    w_theta = np.asarray(w_theta, dtype=np.float32)
    w_phi = np.asarray(w_phi, dtype=np.float32)
    w_g = np.asarray(w_g, dtype=np.float32)
    w_sc = np.asarray(w_sc, dtype=np.float32)
    w_out = np.asarray(w_out, dtype=np.float32)
    b_theta = np.asarray(b_theta, dtype=np.float32)
    b_phi = np.asarray(b_phi, dtype=np.float32)
    b_g = np.asarray(b_g, dtype=np.float32)
    b_sc = np.asarray(b_sc, dtype=np.float32)
    b_out = np.asarray(b_out, dtype=np.float32)

    # The attention-path biases are structurally zero for this problem; the
    # shortcut/output biases fold into a host-side constant row added at the end.
    assert not b_theta.any() and not b_phi.any() and not b_g.any(), (
        "kernel assumes zero theta/phi/g biases"
    )

    B, H, Wd, Cd = x.shape
    assert (B * H, Wd, Cd) == (BH, W, C)

    m1 = w_phi @ w_theta.T
    wsc_out = w_sc @ w_out[C : 2 * C]
    wpg = np.concatenate([w_phi, w_g], axis=1)
    whv = np.concatenate([w_out[0:C], w_out[2 * C : 3 * C]], axis=1)
    ident = np.eye(C, dtype=np.float32)
    bias_row = b_out + b_sc @ w_out[C : 2 * C]  # exact fold of b_sc and b_out

    variant = VARIANT
    np_dt = ml_dtypes.bfloat16 if variant == "bf16" else np.float32
    xs = x.reshape(BH, W, C).astype(np_dt)
    consts = {
        "ident": ident.astype(np_dt),
        "m1": m1.astype(np_dt),
        "wpg": wpg.astype(np_dt),
        "wsc": wsc_out.astype(np_dt),
        "whv": whv.astype(np_dt),
    }
    in_maps = [
        {"x": np.ascontiguousarray(xs[i * SPC : (i + 1) * SPC]), **consts}
        for i in range(N_CORES)
    ]

    nc = _get_nc(variant, REPEAT)
    try:
        last_results = run_bass_kernel_spmd(
            nc, in_maps, core_ids=list(range(N_CORES)), trace=TRACE
        )
    except ModuleNotFoundError:
        # axon NTFF profiling hook unavailable in this environment
        last_results = run_bass_kernel_spmd(
            nc, in_maps, core_ids=list(range(N_CORES)), trace=False
        )
    out = np.concatenate(
        [last_results.results[i]["out"] for i in range(N_CORES)], axis=0
    ).reshape(B, H, W, C)
    if bias_row.any():
        out = out + bias_row
    return out.astype(np.float32)
